# revision 34
# baseline (speedup 1.0000x reference)
"""Trainium2 Bass kernel for nn_Brown: masked directional pixel scatter + 3x3 avg.

Semantics (per image, last two dims H, W):
  pos  = prob <= 20
  avg  = 3x3 reflect-padded box mean of input
  for d in 0..7 sequentially (OFFSETS below):
      m = (dir == d) & pos
      if d == 4: x[m] = avg[m]
      else:      x[q + OFF] = input[q] for masked q (target in range),
                 then x[q] = avg[q] (for q with valid target)

Kernel formulation (validated vs reference):
  key  = (dir+1) * (prob <= 20)                  in {0..8}   (int16)
  Z    = key, zeroed where the self-target is out of range   (int16)
  out  = input copy; self-write first: out = avg where Z != 0
  for d ascending (d != 4), target rectangle p = q + OFF in range:
      u_d = relu((d+1) - Z)                  (ACT; !=0 iff Z < d+1)
      m_d = (key[q] == d+1) * u_d            (STT; !=0 iff neighbor-write wins)
      out[p] = input[q] where m_d != 0       (copy_predicated)
  Ascending overwrite order resolves neighbor-vs-neighbor priority; the
  Z-blocking term resolves self-vs-neighbor priority exactly.

v2: 16-bit datapath. dir/prob are DMA-cast int32->int16 on load, input is
DMA-cast f32->bf16, out is computed in bf16 and DMA-cast back to f32 on
store (gpsimd-initiated DMAs cast in flight). 16-bit tensor ops run at 2x
on DVE; the relu block-terms run on the otherwise-idle ACT engine; the
3x3 average runs on the idle PE as 9 identity-matmul PSUM accumulations
over reflect-halo'd input (exact f32 sums), extracted+scaled by ACT.
bf16 rounding is ~2^-9 relative, far inside the 2e-2 gate.

Sharding: fully data-parallel on batch, 4 batches per core x 8 cores.
"""

import numpy as np

import concourse.bass as bass
import concourse.bacc as bacc
import concourse.mybir as mybir
from concourse import tile
from concourse import bass_utils
from concourse.masks import make_identity

AL = mybir.AluOpType
AF = mybir.ActivationFunctionType
DT = mybir.dt

B, C, H, W = 32, 64, 128, 128
N_CORES = 8
PB = B // N_CORES          # batches per core
NIMG = PB * C              # images per core
NGRP = NIMG // 128         # partition groups of 128 images
R = 32                     # strip rows
NSTRIP = H // R
P_THRESH = 20

# direction -> (di, dj); d=4 is the self (avg-only) case
OFFSETS = {0: (-1, -1), 1: (-1, 0), 2: (-1, 1), 3: (0, -1),
           5: (0, 1), 6: (1, -1), 7: (1, 0)}


def _register_consts(nc, values, dtype=DT.float32):
    for v in values:
        if (dtype, v) in nc.const_aps.aps:
            continue
        t = nc.alloc_sbuf_tensor(f"const-{dtype.name}-{v}", [128, 1], dtype)
        nc.gpsimd.memset(t.ap(), v)
        nc.const_aps.aps[(dtype, v)] = t.ap()
    nc.all_engine_barrier()


def build_brown(nc: bass.Bass, repeat: int = 1, variant: str = 'full'):
    """Emit the full per-core kernel into nc (one SPMD program)."""
    f32, i32 = DT.float32, DT.int32
    _register_consts(nc, [20.5] + [float(d) for d in range(1, 9)])
    inp = nc.dram_tensor("input", [PB, C, H, W], f32, kind="ExternalInput") \
            .ap().rearrange("b c h w -> (b c) h w")
    drm = nc.dram_tensor("dir", [PB, C, H, W], i32, kind="ExternalInput") \
            .ap().rearrange("b c h w -> (b c) h w")
    prm = nc.dram_tensor("prob", [PB, C, H, W], i32, kind="ExternalInput") \
            .ap().rearrange("b c h w -> (b c) h w")
    orm = nc.dram_tensor("out", [PB, C, H, W], f32, kind="ExternalOutput") \
            .ap().rearrange("b c h w -> (b c) h w")

    with tile.TileContext(nc) as tc:
        with tc.tile_pool(name="io", bufs=2) as pio, \
             tc.tile_pool(name="mk", bufs=2) as pmk, \
             tc.tile_pool(name="cst", bufs=1) as pcst, \
             tc.tile_pool(name="ps", bufs=2, space="PSUM") as pps:
            ident = pcst.tile([128, 128], DT.bfloat16, tag="ident")
            make_identity(nc, ident[:])
            if repeat == 0:     # overhead-measurement variant: minimal work
                z = pio.tile([128, W], f32, tag="x")
                nc.sync.dma_start(z[:], inp[0:128, 0, :])
                nc.sync.dma_start(orm[0:128, 0, :], z[:])
            for _ in range(repeat):
                for g in range(NGRP):
                    # split the first strip (pipeline-fill ramp) and the very
                    # last strip (drain tail); all other strips are R rows
                    sched = [(r, R) for r in range(0, H, R)]
                    if g == 0:
                        sched = [(0, R // 2), (R // 2, R // 2)] + sched[1:]
                    if g == NGRP - 1:
                        sched = sched[:-1] + [(H - R, R // 2),
                                              (H - R // 2, R // 2)]
                    for r0, nr in sched:
                        _strip(nc, pio, pmk, pps, ident,
                               inp, drm, prm, orm, g, r0, nr, variant)
    return nc


def _strip(nc, pio, pmk, pps, ident, inp, drm, prm, orm, g, r0, nr,
           variant='full'):
    """One [128 images x nr rows] strip at image row r0. Tiles are sized for
    R rows; a shorter strip (nr < R) just uses the leading rows. Tile row h
    <-> image row r0-1+h; xb tile col c+1 <-> image col c."""
    bf16, i16 = DT.bfloat16, DT.int16
    R_, RH = nr, nr + 2
    isl = slice(g * 128, (g + 1) * 128)
    first, last = (r0 == 0), (r0 + nr == H)

    xb = pio.tile([128, R + 2, W + 2], bf16, tag="x", bufs=3)
    dr = pio.tile([128, R + 2, W], i16, tag="dr", bufs=2)
    pr = pio.tile([128, R + 2, W], i16, tag="pr", bufs=2)

    # ---- casting loads (halo rows: reflect for input; dir/prob halo via memset)
    xc = xb[:, :, 1:W + 1]   # image-aligned column view
    # dir/prob issue first: the key->Z->mask chain is the longest dependency
    # path, so its loads go ahead of input on the in-order SWDGE queue
    if first:
        nc.gpsimd.dma_start(dr[:, 1:R_ + 2, :], drm[isl, 0:R_ + 1, :])
        nc.gpsimd.dma_start(pr[:, 1:R_ + 2, :], prm[isl, 0:R_ + 1, :])
        nc.gpsimd.dma_start(xc[:, 1:R_ + 2, :], inp[isl, 0:R_ + 1, :])
        nc.gpsimd.dma_start(xc[:, 0:1, :], inp[isl, 1:2, :])   # reflect -1 -> 1
    elif last:
        nc.gpsimd.dma_start(dr[:, 0:R_ + 1, :], drm[isl, r0 - 1:H, :])
        nc.gpsimd.dma_start(pr[:, 0:R_ + 1, :], prm[isl, r0 - 1:H, :])
        nc.gpsimd.dma_start(xc[:, 0:R_ + 1, :], inp[isl, r0 - 1:H, :])
        nc.gpsimd.dma_start(xc[:, R_ + 1:R_ + 2, :], inp[isl, H - 2:H - 1, :])
    else:
        nc.gpsimd.dma_start(dr[:, 0:RH, :], drm[isl, r0 - 1:r0 + R_ + 1, :])
        nc.gpsimd.dma_start(pr[:, 0:RH, :], prm[isl, r0 - 1:r0 + R_ + 1, :])
        nc.gpsimd.dma_start(xc[:, 0:RH, :], inp[isl, r0 - 1:r0 + R_ + 1, :])
    # reflect halo columns: tile col 0 <- image col 1; col W+1 <- image col W-2
    nc.vector.tensor_copy(xb[:, 0:RH, 0:1], xb[:, 0:RH, 2:3])
    nc.vector.tensor_copy(xb[:, 0:RH, W + 1:W + 2], xb[:, 0:RH, W - 1:W])

    # ---- key = (dir+1) * sign(20.5 - prob)  in {-8..-1, 1..8}  (int16)
    v0, v1 = (1 if first else 0), (R_ + 1 if last else R_ + 2)   # loaded rows
    vs = slice(v0, v1)
    key = pmk.tile([128, R + 2, W], i16, tag="key")
    pos = pmk.tile([128, R + 2, W], i16, tag="pos", bufs=2)
    nc.scalar.activation(key[:, vs, :], dr[:, vs, :], AF.Identity,
                         bias=1.0, scale=1.0)
    nc.scalar.activation(pos[:, vs, :], pr[:, vs, :], AF.Sign,
                         bias=20.5, scale=-1.0)
    nc.vector.tensor_mul(key[:, vs, :], key[:, vs, :], pos[:, vs, :])
    if first:
        nc.gpsimd.memset(key[:, 0:1, :], 0)     # out-of-image halo: no sources
    if last:
        nc.gpsimd.memset(key[:, R_ + 1:R_ + 2, :], 0)

    # ---- Z = relu(key) with out-of-range self-targets zeroed (int16)
    Z = pmk.tile([128, R, W], i16, tag="Z")
    nc.vector.tensor_scalar(Z[:, 0:R_, :], key[:, 1:R_ + 1, :], 0.0, None,
                            AL.max)
    if first:   # image row 0: self-dirs {0,1,2} (keys 1,2,3) invalid -> keep Z>=4
        nc.vector.scalar_tensor_tensor(Z[:, 0:1, :], Z[:, 0:1, :], 4.0,
                                       Z[:, 0:1, :], AL.is_ge, AL.mult)
    if last:    # image row 127: self-dirs {6,7} (keys 7,8) invalid -> keep Z<=6
        nc.vector.scalar_tensor_tensor(Z[:, R_ - 1:R_, :], Z[:, R_ - 1:R_, :],
                                       6.0, Z[:, R_ - 1:R_, :],
                                       AL.is_le, AL.mult)
    # col 0: self-dirs {0,3,6} (keys 1,4,7) invalid
    for k in (1.0, 4.0, 7.0):
        nc.vector.scalar_tensor_tensor(Z[:, 0:R_, 0:1], Z[:, 0:R_, 0:1], k,
                                       Z[:, 0:R_, 0:1], AL.not_equal, AL.mult)
    # col 127: self-dirs {2,5} (keys 3,6) invalid
    for k in (3.0, 6.0):
        nc.vector.scalar_tensor_tensor(Z[:, 0:R_, W - 1:W], Z[:, 0:R_, W - 1:W],
                                       k, Z[:, 0:R_, W - 1:W],
                                       AL.not_equal, AL.mult)

    # ---- avg = 3x3 reflect box mean: 9-point PSUM accumulation on the PE
    # (identity matmuls of shifted APs over the halo'd input), exact f32.
    do_avg = variant not in ("noavg", "min")
    do_scan = variant not in ("noscan", "min")
    avgb = pio.tile([128, R, W], bf16, tag="avg", bufs=2)
    HR = R_ // 2
    RC = 4                                   # rows per matmul: 4*128 = 512 elems
    if do_avg:
        for half in range(2):
            rr = half * HR
            ps = pps.tile([128, R // 2, W], DT.float32, tag="avgp")
            for rc in range(0, HR, RC):
                po = ps[:, rc:rc + RC, :]
                for di in range(3):
                    rs = slice(rr + rc + di, rr + rc + di + RC)
                    for dj in range(3):
                        nc.tensor.matmul(
                            po, ident[:], xb[:, rs, dj:dj + W],
                            start=(di == 0 and dj == 0),
                            stop=(di == 2 and dj == 2))
            nc.scalar.mul(avgb[:, rr:rr + HR, :], ps[:, 0:HR, :], 1.0 / 9.0)

    # ---- out = input with self-write folded in: out = Z ? avg : input
    outt = pio.tile([128, R, W], bf16, tag="outt", bufs=2)
    if do_avg:
        nc.vector.select(outt[:, 0:R_, :], Z[:, 0:R_, :], avgb[:, 0:R_, :],
                         xb[:, 1:R_ + 1, 1:W + 1])
    else:
        nc.vector.tensor_copy(outt[:, 0:R_, :], xb[:, 1:R_ + 1, 1:W + 1])

    # ---- neighbor scan, ascending d
    for d, (di, dj) in (OFFSETS.items() if do_scan else []):
        c0, c1 = max(dj, 0), W + min(dj, 0)      # target col range
        u = pmk.tile([128, R, W], bf16, tag="u")
        nc.scalar.activation(u[:, 0:R_, :], Z[:, 0:R_, :], AF.Relu,
                             bias=float(d + 1), scale=-1.0)
        m = pmk.tile([128, R, W], i16, tag="m", bufs=2)
        nc.vector.scalar_tensor_tensor(
            m[:, 0:R_, c0:c1],
            key[:, 1 - di:1 - di + R_, c0 - dj:c1 - dj], float(d + 1),
            u[:, 0:R_, c0:c1], AL.is_equal, AL.mult)
        nc.vector.copy_predicated(
            outt[:, 0:R_, c0:c1], m[:, 0:R_, c0:c1],
            xb[:, 1 - di:1 - di + R_, c0 - dj + 1:c1 - dj + 1])

    nc.gpsimd.dma_start(orm[isl, r0:r0 + R_, :], outt[:, 0:R_, :])  # ->f32 cast


_CACHE = {}


def _get_nc(repeat: int = 1, variant: str = "full"):
    k = ("nc", repeat, variant)
    if k not in _CACHE:
        nc = bacc.Bacc("TRN2", target_bir_lowering=False, debug=False)
        build_brown(nc, repeat=repeat, variant=variant)
        nc.compile()
        _CACHE[k] = nc
    return _CACHE[k]


def run(input, dir, prob, trace=False, trace_kwargs=None, repeat=1):
    """Shard over batch, run on 8 cores, gather. Returns (out, BassKernelResults)."""
    nc = _get_nc(repeat)
    in_maps = []
    for c in range(N_CORES):
        bs = slice(c * PB, (c + 1) * PB)
        in_maps.append({
            "input": np.ascontiguousarray(input[bs]),
            "dir": np.ascontiguousarray(dir[bs]),
            "prob": np.ascontiguousarray(prob[bs]),
        })
    res = bass_utils.run_bass_kernel_spmd(
        nc, in_maps, core_ids=list(range(N_CORES)),
        trace=trace, **(trace_kwargs or {}))
    out = np.concatenate([res.results[c]["out"] for c in range(N_CORES)], axis=0)
    return out, res


def kernel(input, dir, prob):
    input = np.asarray(input, dtype=np.float32)
    dir = np.asarray(dir, dtype=np.int32)
    prob = np.asarray(prob, dtype=np.int32)
    out, _ = run(input, dir, prob, trace=False)
    return out
